# revision 56
# baseline (speedup 1.0000x reference)
"""Trainium2 Bass kernel for DepST_RNN (dependency-tree GNN message passing).

Contract: kernel(**inputs) takes FULL inputs, returns FULL output
[B, N, NODE+DEP] float32.  One NeuronCore per sentence (B=8 data-parallel).

V2: matmul-only dataflow — zero SWDGE ops on the critical path.
All indirection (edge gather, scatter-mean, provenance) is baked on host
into one-hot / scaled selection matrices, so every per-layer step is a PE
matmul:
  * uniform slot layout: WR slots per relation per layer (SW = R*WR),
    per-core slot assignment is data (ctxg / oh / Sp / ohf), the
    instruction stream is identical across cores (SPMD, no envelopes).
  * ctx pass: relation-major Wc matmuls over ctxg [256, L*SW] -> mc.
  * per layer l: gather child ct = sum_p chist_p.T @ oh[p,l] (l matmuls),
    40 relation matmuls Wd[r] @ ct[:, r-slots], DVE evac fused with mc add,
    PE transposes to slot-rows, scatter matmul chout = Sp_l.T @ msgS
    (mask/count scaling baked into Sp), evac to compact bf16 chist_l.
  * final: childT = sum_p chist_p.T @ ohf_p, overlapping layer 7.
"""

import sys

sys.path.insert(0, "/opt/trn_rl_repo")

from contextlib import ExitStack

import numpy as np
import ml_dtypes

import concourse.bass as bass
import concourse.bacc as bacc
import concourse.mybir as mybir
from concourse import tile
from concourse.bass_utils import run_bass_kernel_spmd

B, L, E, N = 8, 8, 128, 1024
NODE, DEP, R = 256, 128, 40

BF16 = mybir.dt.bfloat16
F32 = mybir.dt.float32

NPBF16 = ml_dtypes.bfloat16


def prep(context, dep_W, heads, tails, rels, mask):
    """Host-side structure + per-core input tensors."""
    ctx_np = np.asarray(context, np.float32)
    W_np = np.asarray(dep_W, np.float32)
    heads = np.asarray(heads)
    tails = np.asarray(tails)
    rels = np.asarray(rels)
    mask_np = np.asarray(mask, np.float32)

    # per-layer relation-slot width (cross-core envelope)
    cnt = np.zeros((B, L, R), np.int64)
    for b in range(B):
        for l in range(L):
            cnt[b, l] = np.bincount(rels[b, l], minlength=R)
    WRl = [int(cnt[:, l, :].max()) for l in range(L)]
    SWl = [R * w for w in WRl]              # slots in layer l
    NTl = [(s + 127) // 128 for s in SWl]   # transpose/scatter chunks
    TWR = sum(WRl)                          # ctxg relation-block width
    cumWR = np.concatenate([[0], np.cumsum(WRl)]).astype(int)
    GW = R * TWR                 # ctxg columns: col = r*TWR + cumWR[l] + j
    ohbase = np.concatenate([[0], np.cumsum([l * SWl[l] for l in range(L)])]).astype(int)
    sptbase = np.concatenate([[0], np.cumsum([nt * 128 for nt in NTl])]).astype(int)
    SWmax = max(SWl)

    st = dict(WRl=WRl, SWl=SWl, NTl=NTl, TWR=TWR, cumWR=cumWR, GW=GW,
              ohbase=ohbase, sptbase=sptbase, SWmax=SWmax)

    # shared weight layouts (relation-chunk-major so DMA chunks pipeline)
    wc_np = np.zeros((128, 2 * R * 128), np.float32)   # (2r + c) blocks
    wd_np = np.zeros((128, R * 128), np.float32)
    for r in range(R):
        for c in range(2):
            wc_np[:, (2 * r + c) * 128:(2 * r + c + 1) * 128] = (
                W_np[r, :, c * 128:(c + 1) * 128].T
            )
        wd_np[:, r * 128:(r + 1) * 128] = W_np[r, :, NODE:].T
    wc_np = wc_np.astype(NPBF16)
    wd_np = wd_np.astype(NPBF16)
    ident_np = np.eye(128, dtype=np.float32)

    in_maps = []
    st["final"] = []            # per-core (provF, urow) for host-side output gather
    for b in range(B):
        # slot assignment + provenance + compact row maps
        slot = np.zeros((L, E), np.int64)
        for l in range(L):
            c = np.zeros(R, np.int64)
            for e in np.argsort(rels[b, l], kind="stable"):
                r = int(rels[b, l, e])
                slot[l, e] = r * WRl[l] + c[r]
                c[r] += 1
        prov = np.full(N, -1, np.int64)
        provs, uidx = [], []
        for l in range(L):
            provs.append(prov.copy())
            hs = sorted(set(heads[b, l].tolist()))
            assert len(hs) <= 128
            uidx.append({h: i for i, h in enumerate(hs)})
            prov[heads[b, l]] = l
        provF = prov

        ctxg = np.zeros((2 * 128, GW), np.float32)
        ohall = np.zeros((128, int(ohbase[L])), np.float32)
        spt = np.zeros((128, int(sptbase[L])), np.float32)
        for l in range(L):
            cv = np.zeros(N, np.float32)
            np.add.at(cv, heads[b, l], mask_np[b, l])
            for e in range(E):
                s = int(slot[l, e])
                g = int(rels[b, l, e]) * TWR + int(cumWR[l]) + (s % WRl[l])
                t, h = int(tails[b, l, e]), int(heads[b, l, e])
                ctxg[:, g] = ctx_np[b, t, :]
                p = int(provs[l][t])
                if p >= 0:
                    ohall[uidx[p][t], int(ohbase[l]) + p * SWl[l] + s] = 1.0
                u = uidx[l][h]
                spt[s % 128, int(sptbase[l]) + (s // 128) * 128 + u] = (
                    mask_np[b, l, e] / max(float(cv[h]), 1.0)
                )
        urow = np.zeros(N, np.int64)
        for n in range(N):
            p = int(provF[n])
            if p >= 0:
                urow[n] = uidx[p][n]
        st["final"].append((provF.copy(), urow))

        in_maps.append(
            dict(
                ctxg=ctxg.astype(NPBF16),
                wc=wc_np,
                wd=wd_np,
                ohall=ohall.astype(NPBF16),
                spt=spt.astype(NPBF16),
                ident=ident_np,
            )
        )
    return st, in_maps


def build(nc, st):
    WRl, SWl, NTl, TWR, GW = st["WRl"], st["SWl"], st["NTl"], st["TWR"], st["GW"]
    cumWR, ohbase, sptbase, SWmax = st["cumWR"], st["ohbase"], st["sptbase"], st["SWmax"]

    d_ctxg = nc.declare_dram_parameter("ctxg", [256, GW], BF16, isOutput=False)
    d_wc = nc.declare_dram_parameter("wc", [128, 2 * R * 128], BF16, isOutput=False)
    d_wd = nc.declare_dram_parameter("wd", [128, R * 128], BF16, isOutput=False)
    d_oh = nc.declare_dram_parameter(
        "ohall", [128, int(ohbase[L])], BF16, isOutput=False
    )
    d_spt = nc.declare_dram_parameter(
        "spt", [128, int(sptbase[L])], BF16, isOutput=False
    )
    d_ident = nc.declare_dram_parameter("ident", [128, 128], F32, isOutput=False)
    d_out = nc.declare_dram_parameter("chout", [128, L * 128], BF16, isOutput=True)

    NG = 4                      # relation groups for DMA/compute pipelining
    RG = R // NG                # relations per group

    with ExitStack() as ctx:
        tc = ctx.enter_context(tile.TileContext(nc))

        pers = ctx.enter_context(tc.tile_pool(name="pers", bufs=1))

        def sb(name, shape, dt):
            return pers.tile(shape, dt, tag=name, name=name)

        ctxg0 = sb("ctxg0", [128, GW], BF16)
        ctxg1 = sb("ctxg1", [128, GW], BF16)
        wc = sb("wc_sb", [128, 2 * R * 128], BF16)
        wd = sb("wd_sb", [128, R * 128], BF16)
        ohsb = sb("oh_sb", [128, int(ohbase[L])], BF16)
        spt = sb("spt_sb", [128, int(sptbase[L])], BF16)
        ident = sb("ident_sb", [128, 128], F32)
        mcsb = sb("mcsb", [128, GW], BF16)
        chist = sb("chist", [128, L * 128], BF16)

        pool = ctx.enter_context(tc.tile_pool(name="work", bufs=2))
        pp_wide = ctx.enter_context(tc.tile_pool(name="ps_wide", bufs=2, space="PSUM"))
        pp_ct = ctx.enter_context(tc.tile_pool(name="ps_ct", bufs=2, space="PSUM"))
        pp_md = ctx.enter_context(tc.tile_pool(name="ps_md", bufs=1, space="PSUM"))
        pp_t = ctx.enter_context(tc.tile_pool(name="ps_t", bufs=2, space="PSUM"))
        pp_ch = ctx.enter_context(tc.tile_pool(name="ps_ch", bufs=1, space="PSUM"))

        # ---- input DMAs, interleaved for pipelining ----
        # ctxg+wc per relation-group so ctx matmuls start early
        # ctx-critical transfers split across both HWDGE queues so ctxg and
        # wc chunks stream concurrently instead of serializing queue slots
        # wc first in each group: the per-tile LDWEIGHTS prefix (~1.7us)
        # needs wc and can then overlap the ctxg transfer
        for g in range(NG):
            aw = g * RG * 2 * 128
            ww = RG * 2 * 128
            half = ww // 2
            nc.sync.dma_start(wc[:, aw:aw + half], d_wc[:, aw:aw + half])
            nc.scalar.dma_start(
                wc[:, aw + half:aw + ww], d_wc[:, aw + half:aw + ww]
            )
            a = g * RG * TWR
            w = RG * TWR
            nc.sync.dma_start(ctxg0[:, a:a + w], d_ctxg[0:128, a:a + w])
            nc.scalar.dma_start(ctxg1[:, a:a + w], d_ctxg[128:256, a:a + w])
        nc.sync.dma_start(ident[:, :], d_ident[:, :])
        # layer-0 scatter tables, then wd (layer>=1), then per-layer oh+spt
        nc.sync.dma_start(spt[:, 0:int(sptbase[1])], d_spt[:, 0:int(sptbase[1])])
        for g in range(NG):
            aw = g * RG * 128
            ww = RG * 128
            nc.sync.dma_start(wd[:, aw:aw + ww], d_wd[:, aw:aw + ww])
        for l in range(1, L):
            a, a1 = int(ohbase[l]), int(ohbase[l + 1])
            nc.sync.dma_start(ohsb[:, a:a1], d_oh[:, a:a1])
            a, a1 = int(sptbase[l]), int(sptbase[l + 1])
            nc.sync.dma_start(spt[:, a:a1], d_spt[:, a:a1])

        # ---- ctx pass: relation-major Wc matmuls into rotating PSUM tiles ----
        # psum tile width 512 = 6.4 relation blocks (LWR=80); emit matmuls per
        # (relation, k-chunk) split at tile boundaries.
        NCT = (GW + 511) // 512
        for t in range(NCT):
            t0, t1 = 512 * t, min(512 * (t + 1), GW)
            ps = pp_wide.tile([128, 512], F32, tag="wide", name=f"msgc{t}")
            r_lo, r_hi = t0 // TWR, (t1 - 1) // TWR
            for r in range(r_lo, r_hi + 1):
                a = max(r * TWR, t0)
                bnd = min((r + 1) * TWR, t1)
                if a >= bnd:
                    continue
                for c in (0, 1):
                    src = ctxg0 if c == 0 else ctxg1
                    nc.tensor.matmul(
                        ps[:, a - t0:bnd - t0],
                        wc[:, (2 * r + c) * 128:(2 * r + c + 1) * 128],
                        src[:, a:bnd],
                        start=(c == 0),
                        stop=(c == 1),
                    )
            nc.vector.tensor_copy(mcsb[:, t0:t1], ps[:, 0:t1 - t0])

        # ---- recursion over layers ----
        for l in range(L):
            SW, WR, NT = SWl[l], WRl[l], NTl[l]
            if l > 0:
                # gather child: ct = sum_p chist_p.T @ oh[p, l]
                ctp = pp_ct.tile([128, SWmax], F32, tag="ct", name="ct")
                base = int(ohbase[l])
                for p in range(l):
                    nc.tensor.matmul(
                        ctp[:, 0:SW],
                        chist[:, p * 128:(p + 1) * 128],
                        ohsb[:, base + p * SW:base + (p + 1) * SW],
                        start=(p == 0),
                        stop=(p == l - 1),
                    )
                ctsb = pool.tile([128, SWmax], BF16, tag="ctsb", name="ctsb")
                nc.vector.tensor_copy(ctsb[:, 0:20 * WR], ctp[:, 0:20 * WR])
                nc.vector.tensor_copy(ctsb[:, 20 * WR:SW], ctp[:, 20 * WR:SW])
                # relation matmuls: md[:, r-slots] = Wd[r] @ ct[:, r-slots]
                # split across two PSUM banks so the first-half evac (DVE)
                # overlaps the second-half matmuls (no bank collision)
                md = pp_md.tile([128, SWmax], F32, tag="md", name="md")
                mdb = pp_wide.tile([128, 512], F32, tag="wide", name="mdb")
                for r in range(R):
                    dst = md if r < 20 else mdb
                    c0 = r * WR - (0 if r < 20 else 20 * WR)
                    nc.tensor.matmul(
                        dst[:, c0:c0 + WR],
                        wd[:, r * 128:(r + 1) * 128],
                        ctsb[:, r * WR:(r + 1) * WR],
                        start=True,
                        stop=True,
                    )
            # evac + mc add (mc view: cols r*TWR + cumWR[l] + j), 2 halves
            msum = pool.tile([128, SWmax], F32, tag="msum", name="msum")
            mcv = mcsb[:, :].rearrange("p (r lw) -> p r lw", lw=TWR)[
                :, :, int(cumWR[l]):int(cumWR[l]) + WR
            ]
            msv = msum[:, 0:SW].rearrange("p (r w) -> p r w", w=WR)
            RH = R // 2
            if l > 0:
                for h in range(2):
                    rs = slice(h * RH, (h + 1) * RH)
                    src = md if h == 0 else mdb
                    mdv = src[:, 0:RH * WR].rearrange("p (r w) -> p r w", w=WR)
                    nc.vector.tensor_add(
                        msv[:, rs, :], mdv[:, :, :], mcv[:, rs, :]
                    )
            else:
                # split so chunks 0-1's transposes overlap the ctx-pass tail
                RS = 29
                nc.vector.tensor_copy(msv[:, 0:RS, :], mcv[:, 0:RS, :])
                nc.vector.tensor_copy(msv[:, RS:, :], mcv[:, RS:, :])
            # transpose to slot-rows + scatter matmul
            chp = pp_ch.tile([128, 128], F32, tag="chout", name="chout")
            for t in range(NT):
                c0, c1 = 128 * t, min(128 * (t + 1), SW)
                cw = c1 - c0
                if t == 2 and l > 0:
                    # reuse the md tile's dead tail region (same-bank WAR via
                    # program order) so chunk 2 doesn't wait on the tp-pool
                    # rotation behind the chunk-0 copy
                    tpa = md[0:cw, SWmax - 128:SWmax]
                else:
                    tpt = pp_t.tile([128, 128], F32, tag="tp", name="tp")
                    tpa = tpt[0:cw, :]
                nc.tensor.transpose(tpa, msum[:, c0:c1], ident[:, :])
                msgS = pool.tile([128, 128], BF16, tag=f"msgS{t}", name=f"msgS{t}")
                if t % 2 == 0:
                    nc.scalar.copy(msgS[0:cw, :], tpa)
                else:
                    nc.vector.tensor_copy(msgS[0:cw, :], tpa)
                nc.tensor.matmul(
                    chp[:, :],
                    spt[0:cw, int(sptbase[l]) + t * 128:int(sptbase[l]) + (t + 1) * 128],
                    msgS[0:cw, :],
                    start=(t == 0),
                    stop=(t == NT - 1),
                )
            nc.scalar.copy(chist[:, l * 128:(l + 1) * 128], chp[:, :])
            # stream this layer's output tile now — only the 32KB layer-7
            # tile remains on the tail (host does the provenance selection)
            nc.sync.dma_start(
                d_out[:, l * 128:(l + 1) * 128], chist[:, l * 128:(l + 1) * 128]
            )
    return nc


def run(inputs, trace=False, ncores=B, **kw):
    st, in_maps = prep(**inputs)
    nc = bacc.Bacc()
    build(nc, st)
    nc.finalize()
    res = run_bass_kernel_spmd(nc, in_maps[:ncores], list(range(ncores)), trace=trace, **kw)
    ctx_np = np.asarray(inputs["context"], np.float32)
    out = np.zeros((B, N, NODE + DEP), np.float32)
    for b in range(ncores):
        ch = np.asarray(res.results[b]["chout"]).astype(np.float32)
        ch = ch.reshape(128, L, 128)          # [u, layer, dep]
        provF, urow = st["final"][b]
        out[b, :, :NODE] = ctx_np[b]
        upd = provF >= 0
        out[b, upd, NODE:] = ch[urow[upd], provF[upd], :]
    return out, res


def kernel(**inputs):
    out, _ = run(inputs)
    return out


# revision 57
# speedup vs baseline: 1.0495x; 1.0495x over previous
"""Trainium2 Bass kernel for DepST_RNN (dependency-tree GNN message passing).

Contract: kernel(**inputs) takes FULL inputs, returns FULL output
[B, N, NODE+DEP] float32.  One NeuronCore per sentence (B=8 data-parallel).

V2: matmul-only dataflow — zero SWDGE ops on the critical path.
All indirection (edge gather, scatter-mean, provenance) is baked on host
into one-hot / scaled selection matrices, so every per-layer step is a PE
matmul:
  * uniform slot layout: WR slots per relation per layer (SW = R*WR),
    per-core slot assignment is data (ctxg / oh / Sp / ohf), the
    instruction stream is identical across cores (SPMD, no envelopes).
  * ctx pass: relation-major Wc matmuls over ctxg [256, L*SW] -> mc.
  * per layer l: gather child ct = sum_p chist_p.T @ oh[p,l] (l matmuls),
    40 relation matmuls Wd[r] @ ct[:, r-slots], DVE evac fused with mc add,
    PE transposes to slot-rows, scatter matmul chout = Sp_l.T @ msgS
    (mask/count scaling baked into Sp), evac to compact bf16 chist_l.
  * final: childT = sum_p chist_p.T @ ohf_p, overlapping layer 7.
"""

import sys

sys.path.insert(0, "/opt/trn_rl_repo")

from contextlib import ExitStack

import numpy as np
import ml_dtypes

import concourse.bass as bass
import concourse.bacc as bacc
import concourse.mybir as mybir
from concourse import tile
from concourse.bass_utils import run_bass_kernel_spmd

B, L, E, N = 8, 8, 128, 1024
NODE, DEP, R = 256, 128, 40

BF16 = mybir.dt.bfloat16
F32 = mybir.dt.float32

NPBF16 = ml_dtypes.bfloat16


def prep(context, dep_W, heads, tails, rels, mask):
    """Host-side structure + per-core input tensors."""
    ctx_np = np.asarray(context, np.float32)
    W_np = np.asarray(dep_W, np.float32)
    heads = np.asarray(heads)
    tails = np.asarray(tails)
    rels = np.asarray(rels)
    mask_np = np.asarray(mask, np.float32)

    # per-layer relation-slot width (cross-core envelope)
    cnt = np.zeros((B, L, R), np.int64)
    for b in range(B):
        for l in range(L):
            cnt[b, l] = np.bincount(rels[b, l], minlength=R)
    WRl = [int(cnt[:, l, :].max()) for l in range(L)]
    SWl = [R * w for w in WRl]              # slots in layer l
    NTl = [(s + 127) // 128 for s in SWl]   # transpose/scatter chunks
    TWR = sum(WRl)                          # ctxg relation-block width
    cumWR = np.concatenate([[0], np.cumsum(WRl)]).astype(int)
    GW = R * TWR                 # ctxg columns: col = r*TWR + cumWR[l] + j
    ohbase = np.concatenate([[0], np.cumsum([l * SWl[l] for l in range(L)])]).astype(int)
    sptbase = np.concatenate([[0], np.cumsum([nt * 128 for nt in NTl])]).astype(int)
    SWmax = max(SWl)

    st = dict(WRl=WRl, SWl=SWl, NTl=NTl, TWR=TWR, cumWR=cumWR, GW=GW,
              ohbase=ohbase, sptbase=sptbase, SWmax=SWmax)

    # shared weight layouts (relation-chunk-major so DMA chunks pipeline)
    wc_np = np.zeros((128, 2 * R * 128), np.float32)   # (2r + c) blocks
    wd_np = np.zeros((128, R * 128), np.float32)
    for r in range(R):
        for c in range(2):
            wc_np[:, (2 * r + c) * 128:(2 * r + c + 1) * 128] = (
                W_np[r, :, c * 128:(c + 1) * 128].T
            )
        wd_np[:, r * 128:(r + 1) * 128] = W_np[r, :, NODE:].T
    wc_np = wc_np.astype(NPBF16)
    wd_np = wd_np.astype(NPBF16)
    ident_np = np.eye(128, dtype=np.float32)

    in_maps = []
    st["final"] = []            # per-core (provF, urow) for host-side output gather
    for b in range(B):
        # slot assignment + provenance + compact row maps
        slot = np.zeros((L, E), np.int64)
        for l in range(L):
            c = np.zeros(R, np.int64)
            for e in np.argsort(rels[b, l], kind="stable"):
                r = int(rels[b, l, e])
                slot[l, e] = r * WRl[l] + c[r]
                c[r] += 1
        prov = np.full(N, -1, np.int64)
        provs, uidx = [], []
        for l in range(L):
            provs.append(prov.copy())
            hs = sorted(set(heads[b, l].tolist()))
            assert len(hs) <= 128
            uidx.append({h: i for i, h in enumerate(hs)})
            prov[heads[b, l]] = l
        provF = prov

        ctxg = np.zeros((2 * 128, GW), np.float32)
        ohall = np.zeros((128, int(ohbase[L])), np.float32)
        spt = np.zeros((128, int(sptbase[L])), np.float32)
        for l in range(L):
            cv = np.zeros(N, np.float32)
            np.add.at(cv, heads[b, l], mask_np[b, l])
            for e in range(E):
                s = int(slot[l, e])
                g = int(rels[b, l, e]) * TWR + int(cumWR[l]) + (s % WRl[l])
                t, h = int(tails[b, l, e]), int(heads[b, l, e])
                ctxg[:, g] = ctx_np[b, t, :]
                p = int(provs[l][t])
                if p >= 0:
                    ohall[uidx[p][t], int(ohbase[l]) + p * SWl[l] + s] = 1.0
                u = uidx[l][h]
                spt[s % 128, int(sptbase[l]) + (s // 128) * 128 + u] = (
                    mask_np[b, l, e] / max(float(cv[h]), 1.0)
                )
        urow = np.zeros(N, np.int64)
        for n in range(N):
            p = int(provF[n])
            if p >= 0:
                urow[n] = uidx[p][n]
        st["final"].append((provF.copy(), urow))

        in_maps.append(
            dict(
                ctxg=ctxg.astype(NPBF16),
                wc=wc_np,
                wd=wd_np,
                ohall=ohall.astype(NPBF16),
                spt=spt.astype(NPBF16),
                ident=ident_np,
            )
        )
    return st, in_maps


def build(nc, st):
    WRl, SWl, NTl, TWR, GW = st["WRl"], st["SWl"], st["NTl"], st["TWR"], st["GW"]
    cumWR, ohbase, sptbase, SWmax = st["cumWR"], st["ohbase"], st["sptbase"], st["SWmax"]

    d_ctxg = nc.declare_dram_parameter("ctxg", [256, GW], BF16, isOutput=False)
    d_wc = nc.declare_dram_parameter("wc", [128, 2 * R * 128], BF16, isOutput=False)
    d_wd = nc.declare_dram_parameter("wd", [128, R * 128], BF16, isOutput=False)
    d_oh = nc.declare_dram_parameter(
        "ohall", [128, int(ohbase[L])], BF16, isOutput=False
    )
    d_spt = nc.declare_dram_parameter(
        "spt", [128, int(sptbase[L])], BF16, isOutput=False
    )
    d_ident = nc.declare_dram_parameter("ident", [128, 128], F32, isOutput=False)
    d_out = nc.declare_dram_parameter("chout", [128, L * 128], BF16, isOutput=True)

    NG = 4                      # relation groups for DMA/compute pipelining
    RG = R // NG                # relations per group

    with ExitStack() as ctx:
        tc = ctx.enter_context(tile.TileContext(nc))

        pers = ctx.enter_context(tc.tile_pool(name="pers", bufs=1))

        def sb(name, shape, dt):
            return pers.tile(shape, dt, tag=name, name=name)

        ctxg0 = sb("ctxg0", [128, GW], BF16)
        ctxg1 = sb("ctxg1", [128, GW], BF16)
        wc = sb("wc_sb", [128, 2 * R * 128], BF16)
        wd = sb("wd_sb", [128, R * 128], BF16)
        ohsb = sb("oh_sb", [128, int(ohbase[L])], BF16)
        spt = sb("spt_sb", [128, int(sptbase[L])], BF16)
        ident = sb("ident_sb", [128, 128], F32)
        mcsb = sb("mcsb", [128, GW], BF16)
        chist = sb("chist", [128, L * 128], BF16)

        pool = ctx.enter_context(tc.tile_pool(name="work", bufs=2))
        pp_wide = ctx.enter_context(tc.tile_pool(name="ps_wide", bufs=2, space="PSUM"))
        pp_ct = ctx.enter_context(tc.tile_pool(name="ps_ct", bufs=2, space="PSUM"))
        pp_md = ctx.enter_context(tc.tile_pool(name="ps_md", bufs=1, space="PSUM"))
        pp_t = ctx.enter_context(tc.tile_pool(name="ps_t", bufs=2, space="PSUM"))
        pp_ch = ctx.enter_context(tc.tile_pool(name="ps_ch", bufs=1, space="PSUM"))

        # ---- input DMAs, interleaved for pipelining ----
        # ctxg+wc per relation-group so ctx matmuls start early
        # ctx-critical transfers split across both HWDGE queues so ctxg and
        # wc chunks stream concurrently instead of serializing queue slots
        for g in range(NG):
            a = g * RG * TWR
            w = RG * TWR
            nc.sync.dma_start(ctxg0[:, a:a + w], d_ctxg[0:128, a:a + w])
            nc.scalar.dma_start(ctxg1[:, a:a + w], d_ctxg[128:256, a:a + w])
            aw = g * RG * 2 * 128
            ww = RG * 2 * 128
            half = ww // 2
            nc.sync.dma_start(wc[:, aw:aw + half], d_wc[:, aw:aw + half])
            nc.scalar.dma_start(
                wc[:, aw + half:aw + ww], d_wc[:, aw + half:aw + ww]
            )
        nc.sync.dma_start(ident[:, :], d_ident[:, :])
        # layer-0 scatter tables, then wd (layer>=1), then per-layer oh+spt
        nc.sync.dma_start(spt[:, 0:int(sptbase[1])], d_spt[:, 0:int(sptbase[1])])
        for g in range(NG):
            aw = g * RG * 128
            ww = RG * 128
            nc.sync.dma_start(wd[:, aw:aw + ww], d_wd[:, aw:aw + ww])
        for l in range(1, L):
            a, a1 = int(ohbase[l]), int(ohbase[l + 1])
            nc.sync.dma_start(ohsb[:, a:a1], d_oh[:, a:a1])
            a, a1 = int(sptbase[l]), int(sptbase[l + 1])
            nc.sync.dma_start(spt[:, a:a1], d_spt[:, a:a1])

        # ---- ctx pass: relation-major Wc matmuls into rotating PSUM tiles ----
        # psum tile width 512 = 6.4 relation blocks (LWR=80); emit matmuls per
        # (relation, k-chunk) split at tile boundaries.
        NCT = (GW + 511) // 512
        for t in range(NCT):
            t0, t1 = 512 * t, min(512 * (t + 1), GW)
            ps = pp_wide.tile([128, 512], F32, tag="wide", name=f"msgc{t}")
            r_lo, r_hi = t0 // TWR, (t1 - 1) // TWR
            for r in range(r_lo, r_hi + 1):
                a = max(r * TWR, t0)
                bnd = min((r + 1) * TWR, t1)
                if a >= bnd:
                    continue
                for c in (0, 1):
                    src = ctxg0 if c == 0 else ctxg1
                    nc.tensor.matmul(
                        ps[:, a - t0:bnd - t0],
                        wc[:, (2 * r + c) * 128:(2 * r + c + 1) * 128],
                        src[:, a:bnd],
                        start=(c == 0),
                        stop=(c == 1),
                    )
            nc.vector.tensor_copy(mcsb[:, t0:t1], ps[:, 0:t1 - t0])

        # ---- recursion over layers ----
        for l in range(L):
            SW, WR, NT = SWl[l], WRl[l], NTl[l]
            if l > 0:
                # gather child: ct = sum_p chist_p.T @ oh[p, l]
                ctp = pp_ct.tile([128, SWmax], F32, tag="ct", name="ct")
                base = int(ohbase[l])
                for p in range(l):
                    nc.tensor.matmul(
                        ctp[:, 0:SW],
                        chist[:, p * 128:(p + 1) * 128],
                        ohsb[:, base + p * SW:base + (p + 1) * SW],
                        start=(p == 0),
                        stop=(p == l - 1),
                    )
                ctsb = pool.tile([128, SWmax], BF16, tag="ctsb", name="ctsb")
                nc.vector.tensor_copy(ctsb[:, 0:20 * WR], ctp[:, 0:20 * WR])
                nc.vector.tensor_copy(ctsb[:, 20 * WR:SW], ctp[:, 20 * WR:SW])
                # relation matmuls: md[:, r-slots] = Wd[r] @ ct[:, r-slots]
                # split across two PSUM banks so the first-half evac (DVE)
                # overlaps the second-half matmuls (no bank collision)
                md = pp_md.tile([128, SWmax], F32, tag="md", name="md")
                mdb = pp_wide.tile([128, 512], F32, tag="wide", name="mdb")
                for r in range(R):
                    dst = md if r < 20 else mdb
                    c0 = r * WR - (0 if r < 20 else 20 * WR)
                    nc.tensor.matmul(
                        dst[:, c0:c0 + WR],
                        wd[:, r * 128:(r + 1) * 128],
                        ctsb[:, r * WR:(r + 1) * WR],
                        start=True,
                        stop=True,
                    )
            # evac + mc add (mc view: cols r*TWR + cumWR[l] + j), 2 halves
            msum = pool.tile([128, SWmax], F32, tag="msum", name="msum")
            mcv = mcsb[:, :].rearrange("p (r lw) -> p r lw", lw=TWR)[
                :, :, int(cumWR[l]):int(cumWR[l]) + WR
            ]
            msv = msum[:, 0:SW].rearrange("p (r w) -> p r w", w=WR)
            RH = R // 2
            if l > 0:
                for h in range(2):
                    rs = slice(h * RH, (h + 1) * RH)
                    src = md if h == 0 else mdb
                    mdv = src[:, 0:RH * WR].rearrange("p (r w) -> p r w", w=WR)
                    nc.vector.tensor_add(
                        msv[:, rs, :], mdv[:, :, :], mcv[:, rs, :]
                    )
            else:
                # split so chunks 0-1's transposes overlap the ctx-pass tail
                RS = 29
                nc.vector.tensor_copy(msv[:, 0:RS, :], mcv[:, 0:RS, :])
                nc.vector.tensor_copy(msv[:, RS:, :], mcv[:, RS:, :])
            # transpose to slot-rows + scatter matmul
            chp = pp_ch.tile([128, 128], F32, tag="chout", name="chout")
            for t in range(NT):
                c0, c1 = 128 * t, min(128 * (t + 1), SW)
                cw = c1 - c0
                if t == 2 and l > 0:
                    # reuse the md tile's dead tail region (same-bank WAR via
                    # program order) so chunk 2 doesn't wait on the tp-pool
                    # rotation behind the chunk-0 copy
                    tpa = md[0:cw, SWmax - 128:SWmax]
                else:
                    tpt = pp_t.tile([128, 128], F32, tag="tp", name="tp")
                    tpa = tpt[0:cw, :]
                nc.tensor.transpose(tpa, msum[:, c0:c1], ident[:, :])
                msgS = pool.tile([128, 128], BF16, tag=f"msgS{t}", name=f"msgS{t}")
                if t % 2 == 0:
                    nc.scalar.copy(msgS[0:cw, :], tpa)
                else:
                    nc.vector.tensor_copy(msgS[0:cw, :], tpa)
                nc.tensor.matmul(
                    chp[:, :],
                    spt[0:cw, int(sptbase[l]) + t * 128:int(sptbase[l]) + (t + 1) * 128],
                    msgS[0:cw, :],
                    start=(t == 0),
                    stop=(t == NT - 1),
                )
            nc.scalar.copy(chist[:, l * 128:(l + 1) * 128], chp[:, :])
            # stream this layer's output tile now — only the 32KB layer-7
            # tile remains on the tail (host does the provenance selection)
            nc.sync.dma_start(
                d_out[:, l * 128:(l + 1) * 128], chist[:, l * 128:(l + 1) * 128]
            )
    return nc


def run(inputs, trace=False, ncores=B, **kw):
    st, in_maps = prep(**inputs)
    nc = bacc.Bacc()
    build(nc, st)
    nc.finalize()
    res = run_bass_kernel_spmd(nc, in_maps[:ncores], list(range(ncores)), trace=trace, **kw)
    ctx_np = np.asarray(inputs["context"], np.float32)
    out = np.zeros((B, N, NODE + DEP), np.float32)
    for b in range(ncores):
        ch = np.asarray(res.results[b]["chout"]).astype(np.float32)
        ch = ch.reshape(128, L, 128)          # [u, layer, dep]
        provF, urow = st["final"][b]
        out[b, :, :NODE] = ctx_np[b]
        upd = provF >= 0
        out[b, upd, NODE:] = ch[urow[upd], provF[upd], :]
    return out, res


def kernel(**inputs):
    out, _ = run(inputs)
    return out
